# revision 23
# baseline (speedup 1.0000x reference)
"""Trainium2 Bass kernel for a ReActNet-style BasicBlock (binary CNN block).

v4.1: output-half (m) split pipeline with explicit emission ordering.
Engine queues are FIFO, so overlap requires program-order placement:
the stats combine + AllReduce trigger for a half is emitted as soon as
that half's stats exist, while the post-AR affine math is emitted just
before its first consumer.  Small helper DMAs ride on otherwise-idle
queues (cci on ACT's HWDGE ring, gst + output stores on GpSimd SWDGE)
so they cannot head-of-line-block the x loads on the Sync ring.

  A:  [p01 m0][p01 m1][p23 m0]           [p23 m1]
                               \AR1a ......... \AR1b
  B:                                  [B-m0 elementwise][B-m1 + 1x1-m0
      stats][1x1-m1 stats]                       AR2a hides here
  C:  [C-m0: 1x1 recompute + stt + prelu + b23 + store][C-m1]
                                  AR2b hides under C-m0

The 1x1 conv runs twice (stats pass in B, value pass in C into a 7-bank
PSUM tile read by one full-width stt) — PE is idle there and this keeps
c2 out of SBUF entirely.

Numerics: activations +-0.5 fp8 (exact DoubleRow matmuls), SyncBN via
sum/sumsq AllReduce with host-refolded eps, prelu on ACT, fp16 output
cast to fp32 in the store DMA.
Sharding: data-parallel over batch N (32 -> 4 images per core on 8 cores).
"""

import sys

sys.path.insert(0, "/opt/trn_rl_repo")

from contextlib import ExitStack

import numpy as np

import concourse.bass as bass
import concourse.tile as tile
from concourse import bacc, mybir
from concourse.bass_utils import run_bass_kernel_spmd

FP32 = mybir.dt.float32
FP16 = mybir.dt.float16
FP8 = mybir.dt.float8e4
AL = mybir.AluOpType
AF = mybir.ActivationFunctionType
DR = mybir.MatmulPerfMode.DoubleRow

N, C, H, W = 32, 256, 56, 56
PIX = H * W  # 3136
NCORES = 8
NSH = N // NCORES  # images per core
PADW = H + 2  # 58
PADPIX = PADW * PADW  # 3364
GUARD = 4
A1F = 3376  # padded-image span per half; 16-aligned (DoubleRow rhs pair-step)
NCHUNK1 = 7  # 7 chunks x 8 padded rows x 58 cols = 464 free each
CH1 = 464
C1F = NCHUNK1 * CH1  # c1 stored padded (rows 1..56 x 58 cols) = 3248
NCHUNK2 = 7  # 7 chunks x 448 pixels (dense) for the 1x1 conv
CH2 = 448
NTOT = float(N * PIX)  # BN population per channel
EPS = 1e-5

(
    P_NB11, P_P1, P_NT2, P_B1B, P_B23, P_B2B, P_G1, P_G2, P_EPS1, P_EPS2, P_P2,
) = range(11)
NP = 11

TAP_OFF = [58 * (dy - 1) + (dx - 1) for dy in range(3) for dx in range(3)]


def build_nc(nsh=NSH, ncores=NCORES, use_cc=True):
    nc = bacc.Bacc(
        "TRN2", target_bir_lowering=False, debug=False, num_devices=ncores
    )
    nc._use_cc = use_cc
    x_d = nc.dram_tensor("x", [nsh, C, PIX], FP32, kind="ExternalInput")
    w1_d = nc.dram_tensor("w1s", [128, 9 * 2 * 2 * 128], FP8, kind="ExternalInput")
    w2_d = nc.dram_tensor("w2s", [128, 2 * 2 * 128], FP8, kind="ExternalInput")
    pv_d = nc.dram_tensor("pv", [128, 2 * NP], FP32, kind="ExternalInput")
    out_d = nc.dram_tensor("out", [nsh, C, PIX], FP32, kind="ExternalOutput")
    group = [list(range(ncores))]

    with tile.TileContext(nc) as tc, ExitStack() as ctx:
        wp = ctx.enter_context(tc.tile_pool(name="wp", bufs=1))
        stp = ctx.enter_context(tc.tile_pool(name="stp", bufs=1))
        smp = ctx.enter_context(tc.tile_pool(name="smp", bufs=1))
        cu = ctx.enter_context(tc.tile_pool(name="cu", bufs=10))
        a2p = ctx.enter_context(tc.tile_pool(name="a2p", bufs=nsh))
        drp = ctx.enter_context(tc.tile_pool(name="drp", bufs=1, space="DRAM"))

        pvt = wp.tile([128, 2, NP], FP32)
        nc.sync.dma_start(pvt[:], pv_d.ap().rearrange("p (h k) -> p h k", h=2))
        w1t = wp.tile([128, 9, 2, 2, 128], FP8)
        nc.gpsimd.dma_start(w1t[:], w1_d.ap().rearrange("p (t j m o) -> p t j m o", t=9, j=2, m=2))
        w2t = wp.tile([128, 2, 2, 128], FP8)
        nc.gpsimd.dma_start(w2t[:], w2_d.ap().rearrange("p (j m o) -> p j m o", j=2, m=2))

        def pvs(m, k):  # per-partition scalar [128,1] for half m, param k
            return pvt[:, m, k : k + 1]

        sb1 = stp.tile([128, 2, nsh, NCHUNK1, 6], FP32)
        gb1 = stp.tile([128, 2, nsh, 2, 6], FP32)
        sb2 = stp.tile([128, 2, nsh, NCHUNK2, 6], FP32)

        c1 = {}
        u = {}
        a1 = {}
        a2 = {}

        xp_cm = tc.tile_pool(name="xp", bufs=2)
        xp = xp_cm.__enter__()
        a1p_cm = tc.tile_pool(name="a1p", bufs=4)
        a1p = a1p_cm.__enter__()
        hp_cm = tc.tile_pool(name="hp", bufs=2)
        hp = hp_cm.__enter__()
        psp_cm = tc.tile_pool(name="psp", bufs=8, space="PSUM")
        psp = psp_cm.__enter__()

        # =================== phase A: sign -> conv3x3 -> stats ===============
        # pair-grouped, m-split: AR1a (half 0) is triggered right after the
        # last pair's m=0 section and overlaps its m=1 convs.
        gst1 = {}
        pairs = [tuple(range(p, min(p + 2, nsh))) for p in range(0, nsh, 2)]
        for pi, pair in enumerate(pairs):
            for n in pair:
                a1[n] = a1p.tile([128, 2, A1F], FP8, tag="a1", name=f"a1_{n}")
                nc.gpsimd.memset(a1[n][:, :, 0 : GUARD + PADW], 0.0)
                nc.gpsimd.memset(a1[n][:, :, GUARD + 57 * PADW : A1F], 0.0)
                pad_im = a1[n][:, :, GUARD : GUARD + PADPIX].rearrange(
                    "p h (r c) -> p h r c", c=PADW
                )
                nc.gpsimd.memset(pad_im[:, :, 1:57, 0:58:57], 0.0)
                for j in range(2):
                    xh = xp.tile([128, PIX], FP32, tag="x", name="xh")
                    nc.sync.dma_start(
                        xh[:],
                        x_d.ap()[n].rearrange("(h p) f -> p h f", p=128)[:, j, :],
                    )
                    # a1 = (x + b11 >= 0) - 0.5  in {-0.5, +0.5}
                    nc.vector.tensor_scalar(
                        out=pad_im[:, j, 1:57, 1:57],
                        in0=xh[:].rearrange("p (r c) -> p r c", c=W),
                        scalar1=pvs(j, P_NB11),
                        scalar2=0.5,
                        op0=AL.is_ge,
                        op1=AL.subtract,
                    )
            for m in range(2):
                for n in pair:
                    c1[(n, m)] = cu.tile(
                        [128, C1F], FP16, tag="cu", name=f"c1_{n}_{m}"
                    )
                    ps = [
                        psp.tile([128, CH1], FP32, tag="ps", name="ps1")
                        for _ in range(NCHUNK1)
                    ]
                    for t in range(9):
                        lhs = w1t[:, t, :, m, :]
                        for c in range(NCHUNK1):
                            base = GUARD + PADW * (1 + 8 * c) + TAP_OFF[t]
                            nc.tensor.matmul(
                                ps[c][:],
                                lhs,
                                a1[n][:, :, base : base + CH1],
                                start=(t == 0),
                                stop=(t == 8),
                                perf_mode=DR,
                            )
                    for c in range(NCHUNK1):
                        nc.scalar.activation(
                            out=c1[(n, m)][:, CH1 * c : CH1 * (c + 1)],
                            in_=ps[c][:],
                            func=AF.Copy,
                        )
                        nc.vector.bn_stats(out=sb1[:, m, n, c, :], in_=ps[c][:])
                    c1v = c1[(n, m)][:].rearrange("p (r cc) -> p r cc", cc=PADW)
                    nc.vector.bn_stats(out=gb1[:, m, n, 0, :], in_=c1v[:, :, 0])
                    nc.vector.bn_stats(out=gb1[:, m, n, 1, :], in_=c1v[:, :, 57])
                if pi == len(pairs) - 1 and m == 0:
                    gst1[0] = _bn_cc(nc, smp, drp, sb1[:, 0], gb1[:, 0], group, "bn1m0")
        gst1[1] = _bn_cc(nc, smp, drp, sb1[:, 1], gb1[:, 1], group, "bn1m1")

        # ====== phase B: bn1 + residual + prelu + sign2; conv1x1 stats ======
        a1c, b1c, th2 = {}, {}, {}
        a1c[0], b1c[0] = _bn_post(nc, smp, pvt, 0, gst1[0], P_G1, P_B1B, P_EPS1, "bn1m0")
        th2[0] = smp.tile([128, 1], FP32, tag="th2m0", name="th2")
        nc.vector.tensor_tensor(
            out=th2[0][:], in0=pvs(0, P_NT2), in1=b1c[0][:], op=AL.subtract
        )

        def b_work(n, m):
            if m == 0:
                a2[n] = a2p.tile([128, 2, PIX], FP8, tag="a2", name=f"a2_{n}")
            xh = xp.tile([128, PIX], FP32, tag="x", name="xh2")
            nc.sync.dma_start(
                xh[:],
                x_d.ap()[n].rearrange("(h p) f -> p h f", p=128)[:, m, :],
            )
            vr = hp.tile([128, PIX], FP32, tag="h", name="vr")
            c1s = c1[(n, m)][:].rearrange("p (r cc) -> p r cc", cc=PADW)[:, :, 1:57]
            nc.vector.scalar_tensor_tensor(
                out=vr[:].rearrange("p (r c) -> p r c", c=W),
                in0=c1s,
                scalar=a1c[m][:],
                in1=xh[:].rearrange("p (r c) -> p r c", c=W),
                op0=AL.mult,
                op1=AL.add,
            )
            u[(n, m)] = cu.tile([128, PIX], FP16, tag="cu", name=f"u_{n}_{m}")
            nc.scalar.activation(
                out=u[(n, m)][:],
                in_=vr[:],
                func=AF.Prelu,
                bias=b1c[m][:],
                scale=1.0,
                alpha=pvs(m, P_P1),
            )
            nc.vector.tensor_scalar(
                out=a2[n][:, m, :],
                in0=vr[:],
                scalar1=th2[m][:],
                scalar2=0.5,
                op0=AL.is_ge,
                op1=AL.subtract,
            )

        for n in range(nsh):
            b_work(n, 0)

        a1c[1], b1c[1] = _bn_post(nc, smp, pvt, 1, gst1[1], P_G1, P_B1B, P_EPS1, "bn1m1")
        th2[1] = smp.tile([128, 1], FP32, tag="th2m1", name="th2")
        nc.vector.tensor_tensor(
            out=th2[1][:], in0=pvs(1, P_NT2), in1=b1c[1][:], op=AL.subtract
        )

        for n in range(nsh):
            b_work(n, 1)
            # conv1x1 m_out=0 stats as soon as this image's a2 is complete
            _conv1x1_stats(nc, psp, w2t, a2[n], sb2, 0, n)
        gst2 = {}
        gst2[0] = _bn_cc(nc, smp, drp, sb2[:, 0], None, group, "bn2m0")
        for n in range(nsh):
            _conv1x1_stats(nc, psp, w2t, a2[n], sb2, 1, n)
        gst2[1] = _bn_cc(nc, smp, drp, sb2[:, 1], None, group, "bn2m1")

        hp_cm.__exit__(None, None, None)
        a1p_cm.__exit__(None, None, None)
        xp_cm.__exit__(None, None, None)
        psp_cm.__exit__(None, None, None)

        # ========= phase C: conv1x1 recompute + bn2 + prelu + b23 ===========
        with tc.tile_pool(name="pspc", bufs=1, space="PSUM") as pspc, \
             tc.tile_pool(name="wrp", bufs=2) as wrp, \
             tc.tile_pool(name="zp", bufs=2) as zp, \
             tc.tile_pool(name="osb", bufs=4) as osb:
            a2c, b2c = {}, {}
            for m in range(2):
                a2c[m], b2c[m] = _bn_post(
                    nc, smp, pvt, m, gst2[m], P_G2, P_B2B, P_EPS2, f"bn2m{m}"
                )
                lhs = w2t[:, :, m, :]
                for n in range(nsh):
                    psc = pspc.tile([128, NCHUNK2, 512], FP32, tag="psc", name="psc")
                    for c in range(NCHUNK2):
                        nc.tensor.matmul(
                            psc[:, c, 0:CH2],
                            lhs,
                            a2[n][:, :, CH2 * c : CH2 * (c + 1)],
                            start=True,
                            stop=True,
                            perf_mode=DR,
                        )
                    # wraw = alpha2*c2 + u  (beta2 folded into Prelu bias)
                    wr = wrp.tile([128, PIX], FP16, tag="wr", name="wr")
                    nc.vector.scalar_tensor_tensor(
                        out=wr[:].rearrange("p (a b) -> p a b", b=CH2),
                        in0=psc[:, :, 0:CH2],
                        scalar=a2c[m][:],
                        in1=u[(n, m)][:].rearrange("p (a b) -> p a b", b=CH2),
                        op0=AL.mult,
                        op1=AL.add,
                    )
                    z_t = zp.tile([128, PIX], FP16, tag="z", name="z")
                    nc.scalar.activation(
                        out=z_t[:],
                        in_=wr[:],
                        func=AF.Prelu,
                        bias=b2c[m][:],
                        scale=1.0,
                        alpha=pvs(m, P_P2),
                    )
                    o_t = osb.tile([128, PIX], FP16, tag="o", name="o")
                    nc.vector.tensor_scalar_add(
                        out=o_t[:], in0=z_t[:], scalar1=pvs(m, P_B23)
                    )
                    nc.gpsimd.dma_start(
                        out_d.ap()[n].rearrange("(h p) f -> p h f", p=128)[:, m, :],
                        o_t[:],
                    )

    nc.compile()
    return nc


def _conv1x1_stats(nc, psp, w2t, a2n, sb2, m, n):
    lhs = w2t[:, :, m, :]
    for c in range(NCHUNK2):
        ps2 = psp.tile([128, CH2], FP32, tag="ps", name="ps2")
        nc.tensor.matmul(
            ps2[:],
            lhs,
            a2n[:, :, CH2 * c : CH2 * (c + 1)],
            start=True,
            stop=True,
            perf_mode=DR,
        )
        nc.vector.bn_stats(out=sb2[:, m, n, c, :], in_=ps2[:])


def _bn_cc(nc, smp, drp, sb, gb, group, name):
    """Combine one half's bn_stats into (sum, sumsq) and AllReduce.

    Emitted as early as the stats allow; returns the [128,2] gst tile the
    post-AR affine reads.  cci rides the ACT HWDGE ring and gst the GpSimd
    SWDGE ring so neither blocks x loads on the Sync ring.
    """
    loc = smp.tile([128, 2], FP32, tag=f"{name}loc", name="loc")

    def contribs(stats, nt, tag):
        cw = stats.rearrange("p a b (k s) -> p (a b k) s", k=2, s=3)
        t1 = smp.tile([128, nt], FP32, tag=f"{name}{tag}1", name="t1")
        nc.vector.tensor_tensor(
            out=t1[:], in0=cw[:, :, 0], in1=cw[:, :, 1], op=AL.mult
        )
        t2 = smp.tile([128, nt], FP32, tag=f"{name}{tag}2", name="t2")
        nc.vector.tensor_tensor(out=t2[:], in0=t1[:], in1=cw[:, :, 1], op=AL.mult)
        nc.vector.tensor_tensor(out=t2[:], in0=t2[:], in1=cw[:, :, 2], op=AL.add)
        return t1, t2

    nmain = sb.shape[1] * sb.shape[2] * 2
    t1, t2 = contribs(sb, nmain, "s")
    nc.vector.tensor_reduce(
        out=loc[:, 0:1], in_=t1[:], axis=mybir.AxisListType.X, op=AL.add
    )
    nc.vector.tensor_reduce(
        out=loc[:, 1:2], in_=t2[:], axis=mybir.AxisListType.X, op=AL.add
    )
    if gb is not None:
        ng = gb.shape[1] * gb.shape[2] * 2
        g1, g2 = contribs(gb, ng, "g")
        gr = smp.tile([128, 2], FP32, tag=f"{name}gr", name="gr")
        nc.vector.tensor_reduce(
            out=gr[:, 0:1], in_=g1[:], axis=mybir.AxisListType.X, op=AL.add
        )
        nc.vector.tensor_reduce(
            out=gr[:, 1:2], in_=g2[:], axis=mybir.AxisListType.X, op=AL.add
        )
        nc.vector.tensor_tensor(out=loc[:], in0=loc[:], in1=gr[:], op=AL.subtract)

    cci = drp.tile([128, 2], FP32, tag=f"{name}i", name="cci")
    nc.scalar.dma_start(cci[:], loc[:])
    gst = smp.tile([128, 2], FP32, tag=f"{name}g", name="gst")
    if getattr(nc, "_use_cc", True):
        cco = drp.tile([128, 2], FP32, tag=f"{name}o", addr_space="Shared", name="cco")
        nc.gpsimd.collective_compute(
            "AllReduce", AL.add, replica_groups=group, ins=[cci[:]], outs=[cco[:]]
        )
        nc.gpsimd.dma_start(gst[:], cco[:])
    else:
        nc.gpsimd.dma_start(gst[:], cci[:])
    return gst


def _bn_post(nc, smp, pvt, m, gst, kg, kb, keps, name):
    """Post-AllReduce BN affine for one half: [128,1] alpha/beta tiles."""
    mu = smp.tile([128, 1], FP32, tag=f"{name}mu", name="mu")
    nc.vector.tensor_scalar_mul(out=mu[:], in0=gst[:, 0:1], scalar1=1.0 / NTOT)
    var = smp.tile([128, 1], FP32, tag=f"{name}var", name="var")
    nc.vector.tensor_scalar_mul(out=var[:], in0=gst[:, 1:2], scalar1=1.0 / NTOT)
    mu2 = smp.tile([128, 1], FP32, tag=f"{name}mu2", name="mu2")
    nc.vector.tensor_tensor(out=mu2[:], in0=mu[:], in1=mu[:], op=AL.mult)
    nc.vector.tensor_tensor(out=var[:], in0=var[:], in1=mu2[:], op=AL.subtract)
    nc.vector.tensor_tensor(
        out=var[:], in0=var[:], in1=pvt[:, m, keps : keps + 1], op=AL.add
    )
    sig = smp.tile([128, 1], FP32, tag=f"{name}sig", name="sig")
    nc.scalar.activation(out=sig[:], in_=var[:], func=AF.Sqrt)
    inv = smp.tile([128, 1], FP32, tag=f"{name}inv", name="inv")
    nc.vector.reciprocal(out=inv[:], in_=sig[:])
    alpha = smp.tile([128, 1], FP32, tag=f"{name}al", name="alpha")
    nc.vector.tensor_tensor(
        out=alpha[:], in0=inv[:], in1=pvt[:, m, kg : kg + 1], op=AL.mult
    )
    beta = smp.tile([128, 1], FP32, tag=f"{name}be", name="beta")
    nc.vector.tensor_tensor(out=beta[:], in0=mu[:], in1=alpha[:], op=AL.mult)
    nc.vector.tensor_tensor(
        out=beta[:], in0=pvt[:, m, kb : kb + 1], in1=beta[:], op=AL.subtract
    )
    return alpha, beta


def _host_prep(b11, b12, b13, b21, b22, b23, w3x3, wpw, g1, be1, g2, be2, p1, p2):
    f8 = mybir.dt.np(FP8)

    def vec(a):
        return np.asarray(a, np.float32).reshape(C)

    b11, b12, b13 = vec(b11), vec(b12), vec(b13)
    b21, b22, b23 = vec(b21), vec(b22), vec(b23)
    g1, be1, g2, be2, p1, p2 = map(vec, (g1, be1, g2, be2, p1, p2))

    s1 = np.abs(np.asarray(w3x3, np.float32)).mean(axis=(1, 2, 3))
    s2 = np.abs(np.asarray(wpw, np.float32)).mean(axis=(1, 2, 3))

    w1sign = np.sign(np.asarray(w3x3, np.float32)).reshape(2, 128, 2, 128, 9)
    w1s = np.ascontiguousarray(
        np.transpose(w1sign, (3, 4, 2, 0, 1))
    ).astype(f8)  # [p, t, j, m, mo]
    w2sign = np.sign(np.asarray(wpw, np.float32)).reshape(2, 128, 2, 128)
    w2s = np.ascontiguousarray(np.transpose(w2sign, (3, 2, 0, 1))).astype(f8)

    th = -(b13 + b21)
    thv = np.where(th >= 0.0, th, th / np.maximum(p1, 1e-38)).astype(np.float32)
    cols = [
        -b11,                      # P_NB11
        p1,                        # P_P1
        thv,                       # P_NT2 (prelu-inverted sign2 threshold on v)
        be1 + b12,                 # P_B1B
        b23,                       # P_B23
        be2 + b22 + b13,           # P_B2B
        g1,                        # P_G1
        g2,                        # P_G2
        EPS / (4.0 * s1 * s1),     # P_EPS1 (a1 in +-0.5)
        EPS / (4.0 * s2 * s2),     # P_EPS2 (a2 in +-0.5)
        p2,                        # P_P2
    ]
    pv = np.stack(cols, axis=-1).reshape(2, 128, NP)  # [h, p, k]
    pv = np.ascontiguousarray(np.transpose(pv, (1, 0, 2))).astype(np.float32)
    return (
        w1s.reshape(128, 9 * 2 * 2 * 128),
        w2s.reshape(128, 2 * 2 * 128),
        pv.reshape(128, 2 * NP),
    )


_NC_CACHE = {}


def _get_nc():
    if "nc" not in _NC_CACHE:
        _NC_CACHE["nc"] = build_nc()
    return _NC_CACHE["nc"]


def kernel(x, b11, b12, b13, b21, b22, b23, w3x3, wpw, g1, be1, g2, be2, p1, p2):
    nc = _get_nc()
    w1s, w2s, pv = _host_prep(
        b11, b12, b13, b21, b22, b23, w3x3, wpw, g1, be1, g2, be2, p1, p2
    )
    x = np.asarray(x, np.float32).reshape(N, C, PIX)
    in_maps = [
        {
            "x": np.ascontiguousarray(x[c * NSH : (c + 1) * NSH]),
            "w1s": w1s,
            "w2s": w2s,
            "pv": pv,
        }
        for c in range(NCORES)
    ]
    res = run_bass_kernel_spmd(nc, in_maps, core_ids=list(range(NCORES)))
    out = np.concatenate([res.results[c]["out"] for c in range(NCORES)], axis=0)
    return out.reshape(N, C, H, W)


if __name__ == "__main__":
    nc = build_nc()
    print("built + compiled OK; instructions:", sum(
        len(bb.instructions) for bb in nc.m.functions[0].blocks
    ))
